# revision 7
# baseline (speedup 1.0000x reference)
"""Trainium2 Bass kernel for a 16-head causal MHA layer with relative-position
bias (B=2, S=2048, D=1024, H=16, HD=64), distributed over 8 NeuronCores.

Sharding: data parallel over nothing / tensor parallel over heads — core c
computes heads {2c, 2c+1} for both batches.  The output projection is sharded
over its input dim, so each core returns a partial (B, S, D) output; the
partials are summed on the host (plus proj_b).

Per-core device pipeline (all matmuls in float32r, TF32-class precision):
  1. QKV projections in transposed layout: QT/KT/VT (128=2*HD, 2048) from
     XT (D, S) chunks x weight-slice chunks.  K' = K + rpr[positions] folded
     in during the PSUM->SBUF eviction.
  2. Attention per head in S^T layout: scores S^T(j,i) = K'^T-slice . Q-slice
     (both heads packed into one PE pass via tile_position row tiling),
     causal-masked via a tril constant on diagonal blocks, exp on the scalar
     engine (scale folded in), then OT_aug(65, i) += V_aug(j, 65)^T . P^T with
     a ones-column producing the softmax denominators for free.  Fully-masked
     j-blocks are skipped.
  3. Normalize: OT = OT_aug[0:64] * bcast(1/OT_aug[64]).
  4. Output projection: y_partial(s, e) += OT2^T . pwT, streamed to DRAM.
"""

import sys

import numpy as np

try:
    import concourse.bass as bass  # noqa: F401
except ImportError:
    sys.path.insert(0, "/opt/trn_rl_repo")

import concourse.bass as bass
import concourse.mybir as mybir
import concourse.tile as tile
from concourse import bacc
from concourse.bass_utils import run_bass_kernel_spmd

B, S, D, H = 2, 2048, 1024, 16
HD = D // H  # 64
SCALE = HD**-0.5
N_CORES = 8
HPC = H // N_CORES  # heads per core = 2
DL = HPC * HD  # local head channels = 128
NJ = S // 128  # 16 j-chunks of 128
NI = S // 512  # 4 i-blocks of 512
KC = D // 128  # 8 contraction chunks of 128

F32 = mybir.dt.float32
F32R = mybir.dt.float32r

_BUILD_CACHE: dict = {}


def _emit(nc, tc, t, mode, niter):
    """Emit the per-core program.  `t` maps dram tensor name -> handle."""
    xt = t["xt"].ap()  # (B, D, S) f32r
    wT = [t["wqT"].ap(), t["wkT"].ap(), t["wvT"].ap()]  # (D, DL) f32r
    pwT = t["pwT"].ap()  # (DL, D) f32r
    rpr2 = t["rpr2T"].ap()  # (128, S) f32r : b-th 64 rows = rprT for batch b
    tril = t["trilT"].ap()  # (128, 128) f32
    ident = t["ident"].ap()  # (128, 128) f32r
    onesc = t["onesc"].ap()  # (128, 1) f32r
    zeroc = t["zeroc"].ap()  # (128, 384) f32r
    y = t["y"].ap()  # (B, S, D) f32
    maskT = t["maskT"].ap() if "maskT" in t else None  # (S, S) f32

    ctxs = [
        tc.tile_pool(name="consts", bufs=1),
        tc.tile_pool(name="xt", bufs=1),
        tc.tile_pool(name="qkv", bufs=1),
        tc.tile_pool(name="va", bufs=1),
        tc.tile_pool(name="pt", bufs=4),
        tc.tile_pool(name="sm", bufs=2),
        tc.tile_pool(name="ysb", bufs=3),
        tc.tile_pool(name="ps_mm", bufs=2, space="PSUM"),
        tc.tile_pool(name="ps_sc", bufs=3, space="PSUM"),
        tc.tile_pool(name="ps_ot", bufs=2, space="PSUM"),
        tc.tile_pool(name="ps_tr", bufs=1, space="PSUM"),
    ]
    if maskT is not None:
        ctxs.append(tc.tile_pool(name="mk", bufs=4))
    pools = [c.__enter__() for c in ctxs]
    (consts, xtp, qkvp, vap, ptp, smp, ysbp, ps_mm, ps_sc, ps_ot, ps_tr) = pools[:11]
    mkp = pools[11] if maskT is not None else None

    # --- persistent constants ---
    w_t = [[consts.tile([128, DL], F32R, tag=f"w{p}{k}", name=f"w{p}{k}")
            for k in range(KC)] for p in range(3)]
    for p in range(3):
        for k in range(KC):
            nc.sync.dma_start(w_t[p][k][:], wT[p][k * 128:(k + 1) * 128, :])
    pw_t = consts.tile([DL, D], F32R, tag="pw")
    nc.sync.dma_start(pw_t[:], pwT)
    rpr_t = consts.tile([128, S], F32R, tag="rpr")
    nc.sync.dma_start(rpr_t[:], rpr2)
    tril_t = consts.tile([128, 128], F32, tag="tril")
    nc.sync.dma_start(tril_t[:], tril)
    id_t = consts.tile([128, 128], F32R, tag="id")
    nc.sync.dma_start(id_t[:], ident)

    def body(_iv=None):
        for b in range(B):
            # ---- phase 1: load XT, QKV projections (transposed layout) ----
            xt_t = []
            for k in range(KC):
                xtk = xtp.tile([128, S], F32R, tag=f"xt{k}", name=f"xt{k}")
                nc.sync.dma_start(xtk[:], xt[b, k * 128:(k + 1) * 128, :])
                xt_t.append(xtk)

            qt = qkvp.tile([128, S], F32R, tag="qt")
            kt = qkvp.tile([128, S], F32R, tag="kt")
            vt = qkvp.tile([128, S], F32R, tag="vt")
            for p, dst in ((0, qt), (1, kt), (2, vt)):
                for sb in range(NI):
                    ps = ps_mm.tile([128, 512], F32, tag="mm")
                    for k in range(KC):
                        nc.tensor.matmul(
                            ps[:], w_t[p][k][:], xt_t[k][:, sb * 512:(sb + 1) * 512],
                            start=(k == 0), stop=(k == KC - 1))
                    sl = slice(sb * 512, (sb + 1) * 512)
                    if p == 1:
                        # K' = K + rpr (same bias for both head halves)
                        rsl = rpr_t[64 * b:64 * b + 64, sl]
                        nc.vector.tensor_add(dst[0:64, sl], ps[0:64, :], rsl)
                        nc.vector.tensor_add(dst[64:128, sl], ps[64:128, :], rsl)
                    else:
                        nc.vector.tensor_copy(dst[:, sl], ps[:])

            # ---- V transpose to natural layout, with ones columns ----
            va_t = []
            for j in range(NJ):
                tp = ps_tr.tile([128, 128], F32R, tag="tr")
                nc.tensor.transpose(tp[:], vt[:, j * 128:(j + 1) * 128], id_t[:])
                va = vap.tile([128, 2 * (HD + 1)], F32R, tag=f"va{j}")
                nc.vector.tensor_copy(va[:, 0:HD], tp[:, 0:HD])
                nc.vector.tensor_copy(va[:, HD + 1:2 * HD + 1], tp[:, HD:2 * HD])
                nc.sync.dma_start(va[:, HD:HD + 1], onesc)
                nc.sync.dma_start(va[:, 2 * HD + 1:2 * HD + 2], onesc)
                va_t.append(va)

            ot2 = qkvp.tile([128, S], F32R, tag="ot2")

            # ---- phase 2: attention ----
            for it in range(NI):
                isl = slice(it * 512, (it + 1) * 512)
                jhi = (4 * it + 3) if mode == "causal" else (NJ - 1)
                otp = [ps_ot.tile([HD + 1, 512], F32, tag="ot", name=f"ot{_h}")
                       for _h in range(HPC)]
                for j in range(jhi + 1):
                    jsl = slice(j * 128, (j + 1) * 128)
                    sc = [ps_sc.tile([128, 512], F32, tag="sc", name=f"sc{_h}")
                           for _h in range(HPC)]
                    for h in range(HPC):
                        hsl = slice(h * HD, (h + 1) * HD)
                        nc.tensor.matmul(
                            sc[h][:], kt[hsl, jsl], qt[hsl, isl],
                            start=True, stop=True, tile_position=(h * HD, 0))
                    if maskT is not None:
                        mkt = mkp.tile([128, 512], F32, tag="mk")
                        nc.sync.dma_start(mkt[:], maskT[jsl, isl])
                    dc = max(0, (j - 4 * it) * 128) if mode == "causal" else 0
                    for h in range(HPC):
                        pt = ptp.tile([128, 512], F32R, tag="pt")
                        if maskT is not None:
                            nc.vector.tensor_add(sc[h][:], sc[h][:], mkt[:])
                        elif mode == "causal" and j >= 4 * it:
                            nc.vector.tensor_add(
                                sc[h][:, dc:dc + 128], sc[h][:, dc:dc + 128],
                                tril_t[:])
                        if dc > 0:
                            nc.sync.dma_start(pt[:, 0:dc], zeroc[:, 0:dc])
                        nc.scalar.activation(
                            pt[:, dc:512], sc[h][:, dc:512],
                            mybir.ActivationFunctionType.Exp, scale=SCALE)
                        nc.tensor.matmul(
                            otp[h][:], va_t[j][:, h * (HD + 1):(h + 1) * (HD + 1)],
                            pt[:], start=(j == 0), stop=(j == jhi),
                            skip_group_check=True)
                # normalize: OT2[h] = OT_aug[0:64] * bcast(1 / OT_aug[64])
                for h in range(HPC):
                    rec = smp.tile([1, 512], F32R, tag="rec")
                    nc.vector.reciprocal(rec[:], otp[h][HD:HD + 1, :])
                    bc = smp.tile([HD, 512], F32R, tag="bc")
                    nc.gpsimd.partition_broadcast(bc[:], rec[:])
                    nc.vector.tensor_mul(
                        ot2[h * HD:(h + 1) * HD, isl], otp[h][0:HD, :], bc[:])

            # ---- phase 3: output projection (partial over local heads) ----
            for st in range(S // 128):
                ssl = slice(st * 128, (st + 1) * 128)
                yt = ysbp.tile([128, D], F32, tag="y")
                for eb in range(D // 512):
                    pp = ps_mm.tile([128, 512], F32, tag="mm")
                    nc.tensor.matmul(
                        pp[:], ot2[:, ssl], pw_t[:, eb * 512:(eb + 1) * 512],
                        start=True, stop=True)
                    nc.vector.tensor_copy(yt[:, eb * 512:(eb + 1) * 512], pp[:])
                nc.sync.dma_start(y[b, ssl, :], yt[:])

    if niter == 1:
        body()
    else:
        with tc.For_i(0, niter, 1) as iv:
            body(iv)

    for c in reversed(ctxs):
        c.__exit__(None, None, None)


def _build(mode, niter=1):
    key = (mode, niter)
    if key in _BUILD_CACHE:
        return _BUILD_CACHE[key]
    nc = bacc.Bacc("TRN2", target_bir_lowering=False, debug=False,
                   num_devices=N_CORES)
    t = {}
    t["xt"] = nc.dram_tensor("xt", (B, D, S), F32R, kind="ExternalInput")
    t["wqT"] = nc.dram_tensor("wqT", (D, DL), F32R, kind="ExternalInput")
    t["wkT"] = nc.dram_tensor("wkT", (D, DL), F32R, kind="ExternalInput")
    t["wvT"] = nc.dram_tensor("wvT", (D, DL), F32R, kind="ExternalInput")
    t["pwT"] = nc.dram_tensor("pwT", (DL, D), F32R, kind="ExternalInput")
    t["rpr2T"] = nc.dram_tensor("rpr2T", (128, S), F32R, kind="ExternalInput")
    t["trilT"] = nc.dram_tensor("trilT", (128, 128), F32, kind="ExternalInput")
    t["ident"] = nc.dram_tensor("ident", (128, 128), F32R, kind="ExternalInput")
    t["onesc"] = nc.dram_tensor("onesc", (128, 1), F32R, kind="ExternalInput")
    t["zeroc"] = nc.dram_tensor("zeroc", (128, 384), F32R, kind="ExternalInput")
    if mode == "generic":
        t["maskT"] = nc.dram_tensor("maskT", (S, S), F32, kind="ExternalInput")
    t["y"] = nc.dram_tensor("y", (B, S, D), F32, kind="ExternalOutput")

    with tile.TileContext(nc) as tc, \
            nc.allow_low_precision(reason="float32r matmul operands (TF32-class)"):
        _emit(nc, tc, t, mode, niter)
    nc.compile()
    _BUILD_CACHE[key] = (nc, t)
    return nc, t


def _prep_inputs(x, positions, causal_mask, wq, wk, wv, rpr, proj_w):
    """Host-side shard prep.  Returns (mode, per-core input maps)."""
    mask = np.asarray(causal_mask, np.float32).reshape(S, S)
    low = np.tril(np.ones((S, S), dtype=bool))
    if not mask.any():
        mode, mval = "zero", 0.0
    elif (mask[low] == 0.0).all() and np.all(mask[~low] <= -1e6) \
            and np.all(mask[~low] == mask[0, 1]):
        mode, mval = "causal", float(mask[0, 1])
    else:
        mode, mval = "generic", 0.0

    xt = np.ascontiguousarray(np.asarray(x, np.float32).transpose(0, 2, 1))
    pos = np.asarray(positions).astype(np.int64)
    rpr_g = np.asarray(rpr, np.float32)[pos]  # (B, S, HD)
    rpr2 = np.ascontiguousarray(
        rpr_g.transpose(0, 2, 1)).reshape(B * HD, S)  # (128, S)
    jj = np.arange(128)[:, None]
    ii = np.arange(128)[None, :]
    trilT = np.where(jj <= ii, 0.0, mval).astype(np.float32)
    ident = np.eye(128, dtype=np.float32)
    maskT = np.ascontiguousarray(mask.T) if mode == "generic" else None

    wq = np.asarray(wq, np.float32)
    wk = np.asarray(wk, np.float32)
    wv = np.asarray(wv, np.float32)
    pw = np.asarray(proj_w, np.float32)

    in_maps = []
    for c in range(N_CORES):
        rs = slice(c * DL, (c + 1) * DL)
        m = {
            "xt": xt,
            "wqT": np.ascontiguousarray(wq[rs, :].T),
            "wkT": np.ascontiguousarray(wk[rs, :].T),
            "wvT": np.ascontiguousarray(wv[rs, :].T),
            "pwT": np.ascontiguousarray(pw[:, rs].T),
            "rpr2T": rpr2,
            "trilT": trilT,
            "ident": ident,
            "onesc": np.ones((128, 1), np.float32),
            "zeroc": np.zeros((128, 384), np.float32),
        }
        if maskT is not None:
            m["maskT"] = maskT
        in_maps.append(m)
    return mode, in_maps


def kernel(x, positions, causal_mask, wq, wk, wv, rpr, proj_w, proj_b,
           _niter=1, **_ignored):
    mode, in_maps = _prep_inputs(x, positions, causal_mask, wq, wk, wv, rpr,
                                 proj_w)
    nc, _ = _build(mode, _niter)
    res = run_bass_kernel_spmd(nc, in_maps, core_ids=list(range(N_CORES)))
    out = np.zeros((B, S, D), dtype=np.float32)
    for r in res.results:
        out += r["y"]
    out += np.asarray(proj_b, np.float32)[None, None, :]
    return out


# revision 27
# speedup vs baseline: 81.3253x; 81.3253x over previous
"""Trainium2 Bass kernel for a 16-head causal MHA layer with relative-position
bias (B=2, S=2048, D=1024, H=16, HD=64), distributed over 8 NeuronCores.

Sharding: data parallel over nothing / tensor parallel over heads — core c
computes heads {2c, 2c+1} for both batches.  The output projection is sharded
over its input dim, so each core returns a partial (B, S, D) output; the
partials are summed on the host (plus proj_b).

Per-core device pipeline (all matmuls in float32r, TF32-class precision):
  1. QKV projections in transposed layout: QT/KT/VT (128=2*HD, 2048) from
     XT (D, S) chunks x weight-slice chunks.  K' = K + rpr[positions] folded
     in during the PSUM->SBUF eviction.
  2. Attention per head in S^T layout: scores S^T(j,i) = K'^T-slice . Q-slice
     (both heads packed into one PE pass via tile_position row tiling),
     causal-masked via a tril constant on diagonal blocks, exp on the scalar
     engine (scale folded in), then OT_aug(65, i) += V_aug(j, 65)^T . P^T with
     a ones-column producing the softmax denominators for free.  Fully-masked
     j-blocks are skipped.
  3. Normalize: OT = OT_aug[0:64] * bcast(1/OT_aug[64]).
  4. Output projection: y_partial(s, e) += OT2^T . pwT, streamed to DRAM.
"""

import sys

import numpy as np

try:
    import concourse.bass as bass  # noqa: F401
except ImportError:
    sys.path.insert(0, "/opt/trn_rl_repo")

import concourse.bass as bass
import concourse.mybir as mybir
import concourse.tile as tile
from concourse import bacc
from concourse.bass_utils import run_bass_kernel_spmd

B, S, D, H = 2, 2048, 1024, 16
HD = D // H  # 64
SCALE = HD**-0.5
N_CORES = 8
HPC = H // N_CORES  # heads per core = 2
DL = HPC * HD  # local head channels = 128
NJ = S // 128  # 16 j-chunks of 128
NI = S // 512  # 4 i-blocks of 512
KC = D // 128  # 8 contraction chunks of 128

F32 = mybir.dt.float32
F32R = mybir.dt.float32r
F16 = mybir.dt.float16

_BUILD_CACHE: dict = {}


def _emit(nc, tc, t, mode, niter, phases=('p1', 'att', 'proj')):
    """Emit the per-core program.  `t` maps dram tensor name -> handle."""
    xt = t["xt"].ap()  # (B, D, S) f32r
    wT = [t["wqT"].ap(), t["wkT"].ap(), t["wvT"].ap()]  # (D, DL) f32r
    pwT = t["pwT"].ap()  # (DL, D) f32r
    rpr2 = t["rpr2T"].ap()  # (128, S) f32r : b-th 64 rows = rprT for batch b
    tril = t["trilT"].ap()  # (128, 128) f32
    ident = t["ident"].ap()  # (128, 128) f32r
    onesc = t["onesc"].ap()  # (128, 1) f32r
    ones1 = t["ones1"].ap()  # (1, HD)
    zeroc = t["zeroc"].ap()  # (128, 384) f32r
    y = t["y"].ap()  # (B, S, D) f32
    maskT = t["maskT"].ap() if "maskT" in t else None  # (S, S) f32

    ctxs = [
        tc.tile_pool(name="consts", bufs=1),
        tc.tile_pool(name="xt", bufs=2),
        tc.tile_pool(name="qkv", bufs=1),
        tc.tile_pool(name="va", bufs=1),
        tc.tile_pool(name="pt", bufs=3),
        tc.tile_pool(name="sm", bufs=2),
        tc.tile_pool(name="ysb", bufs=3),
        tc.tile_pool(name="ps_mm", bufs=2, space="PSUM"),
        tc.tile_pool(name="ps_sc", bufs=2, space="PSUM"),
        tc.tile_pool(name="ps_ot", bufs=2, space="PSUM"),
    ]
    if maskT is not None:
        ctxs.append(tc.tile_pool(name="mk", bufs=4))
    pools = [c.__enter__() for c in ctxs]
    (consts, xtp, qkvp, vap, ptp, smp, ysbp, ps_mm, ps_sc, ps_ot) = pools[:10]
    mkp = pools[10] if maskT is not None else None

    # --- persistent constants ---
    w_t = [[consts.tile([128, DL], F16, tag=f"w{p}{k}", name=f"w{p}{k}")
            for k in range(KC)] for p in range(3)]
    for p in range(3):
        for k in range(KC):
            nc.sync.dma_start(w_t[p][k][:], wT[p][k * 128:(k + 1) * 128, :])
    pw_t = consts.tile([DL, D], F16, tag="pw")
    nc.sync.dma_start(pw_t[:], pwT)
    rpr_t = consts.tile([128, S], F16, tag="rpr")
    nc.sync.dma_start(rpr_t[:], rpr2)
    tril_t = consts.tile([128, 128], F32, tag="tril")
    nc.sync.dma_start(tril_t[:], tril)
    id_t = consts.tile([128, 128], F16, tag="id")
    nc.sync.dma_start(id_t[:], ident)
    ones_t = consts.tile([128, 1], F16, tag="ones")
    nc.sync.dma_start(ones_t[:], onesc)
    ones1_t = consts.tile([1, HD], F16, tag="ones1")
    nc.sync.dma_start(ones1_t[:], ones1)

    def body(_iv=None):
        if "p1" not in phases:
            # DMA-only skeleton: load xt, store zeros-ish y
            for b in range(B):
                for k in range(KC):
                    xtk = xtp.tile([128, S], F16, tag=f"xt{k}", name=f"xt{k}")
                    nc.sync.dma_start(xtk[:], xt[b, k * 128:(k + 1) * 128, :])
                for st in range(S // 128):
                    yt = ysbp.tile([128, D], F16, tag="y")
                    nc.vector.tensor_copy(yt[:, 0:512], xtk[:, 0:512])
                    nc.vector.tensor_copy(yt[:, 512:1024], xtk[:, 0:512])
                    nc.scalar.dma_start(y[b, st * 128:(st + 1) * 128, :], yt[:])
            return

        xts, qkv, vas, ot2s = {}, {}, {}, {}

        def emit_loads(b):
            xts[b] = []
            for k in range(KC):
                xtk = xtp.tile([128, S], F16, tag=f"xt{k}", name=f"xt{k}")
                nc.sync.dma_start(xtk[:], xt[b, k * 128:(k + 1) * 128, :])
                xts[b].append(xtk)
            qkv[b] = (
                qkvp.tile([128, S], F16, tag=f"qt{b}", name=f"qt{b}"),
                qkvp.tile([128, S], F16, tag=f"kt{b}", name=f"kt{b}"),
                qkvp.tile([128, S], F16, tag=f"vt{b}", name=f"vt{b}"),
            )
            ot2s[b] = qkvp.tile([128, S], F16, tag=f"ot2_{b}", name=f"ot2_{b}")

        def emit_qkv_group(b, g):
            # g in 0..11: projection p = g // NI, 512-col block sb = g % NI
            p, sb = g // NI, g % NI
            dst = qkv[b][p]
            ps = ps_mm.tile([128, 512], F32, tag="mm")
            for k in range(KC):
                nc.tensor.matmul(
                    ps[:], w_t[p][k][:], xts[b][k][:, sb * 512:(sb + 1) * 512],
                    start=(k == 0), stop=(k == KC - 1))
            sl = slice(sb * 512, (sb + 1) * 512)
            if p == 1:
                # K' = K + rpr (same bias for both head halves)
                rsl = rpr_t[64 * b:64 * b + 64, sl]
                nc.vector.tensor_add(dst[0:64, sl], ps[0:64, :], rsl)
                nc.vector.tensor_add(dst[64:128, sl], ps[64:128, :], rsl)
            else:
                nc.vector.tensor_copy(dst[:, sl], ps[:])

        def emit_va(b):
            # V transposed to natural layout, with ones columns appended per
            # head (the ones produce the softmax denominators in the AV pass)
            vas[b] = []
            vt = qkv[b][2]
            for j in range(NJ):
                tp = ps_mm.tile([128, 128], F16, tag="mm", name="tp")
                nc.tensor.transpose(tp[:], vt[:, j * 128:(j + 1) * 128], id_t[:])
                va = vap.tile([128, 2 * (HD + 1)], F16, tag=f"va{b}_{j}",
                              name=f"va{b}_{j}")
                nc.vector.tensor_copy(va[:, 0:HD], tp[:, 0:HD])
                nc.vector.tensor_copy(va[:, HD + 1:2 * HD + 1], tp[:, HD:2 * HD])
                nc.vector.tensor_copy(va[:, HD:HD + 1], ones_t[:])
                nc.vector.tensor_copy(va[:, 2 * HD + 1:2 * HD + 2], ones_t[:])
                vas[b].append(va)

        def emit_proj_blocks(b, sts):
            if "proj" not in phases:
                return
            for st in sts:
                ssl = slice(st * 128, (st + 1) * 128)
                yt = ysbp.tile([128, D], F16, tag="y")
                for eb in range(D // 512):
                    pp = ps_mm.tile([128, 512], F32, tag="mm")
                    nc.tensor.matmul(
                        pp[:], ot2s[b][:, ssl],
                        pw_t[:, eb * 512:(eb + 1) * 512],
                        start=True, stop=True)
                    nc.vector.tensor_copy(
                        yt[:, eb * 512:(eb + 1) * 512], pp[:])
                if "nostore" not in phases:
                    eng = nc.scalar if (st % 2) else nc.sync
                    eng.dma_start(y[b, ssl, :], yt[:])

        def emit_att_it(b, it):
            # scores -> exp -> AV for one 512-wide query block, software-
            # pipelined two j-chunks deep so the in-order PE queue is not
            # gated on the ACT exp latency each chunk.
            qt, kt, _ = qkv[b]
            isl = slice(it * 512, (it + 1) * 512)
            jhi = (4 * it + 3) if mode == "causal" else (NJ - 1)
            otp = [ps_ot.tile([HD + 1, 512], F32, tag="ot", name=f"ot{_h}")
                   for _h in range(HPC)]

            def emit_scores(j):
                jsl = slice(j * 128, (j + 1) * 128)
                sc2 = ps_sc.tile([128, 2 * 512], F32, tag="sc", name="sc2")
                for h in range(HPC):
                    hsl = slice(h * HD, (h + 1) * HD)
                    nc.tensor.matmul(
                        sc2[:, h * 512:(h + 1) * 512], kt[hsl, jsl],
                        qt[hsl, isl], start=True, stop=True,
                        tile_position=(h * HD, 0))
                if maskT is not None:
                    mkt = mkp.tile([128, 512], F32, tag="mk")
                    nc.sync.dma_start(mkt[:], maskT[jsl, isl])
                    for h in range(HPC):
                        nc.vector.tensor_add(
                            sc2[:, h * 512:(h + 1) * 512],
                            sc2[:, h * 512:(h + 1) * 512], mkt[:])
                return sc2

            def emit_exp_av(j, sc2):
                dc = max(0, (j - 4 * it) * 128) if mode == "causal" else 0
                pt2 = ptp.tile([128, 2 * 512], F16, tag="pt", name="pt2")
                if mode == "causal" and j >= 4 * it:
                    for h in range(HPC):
                        nc.vector.tensor_add(
                            sc2[:, h * 512 + dc:h * 512 + dc + 128],
                            sc2[:, h * 512 + dc:h * 512 + dc + 128],
                            tril_t[:])
                if dc == 0:
                    nc.scalar.activation(
                        pt2[:], sc2[:],
                        mybir.ActivationFunctionType.Exp, scale=SCALE)
                else:
                    for h in range(HPC):
                        nc.scalar.activation(
                            pt2[:, h * 512 + dc:(h + 1) * 512],
                            sc2[:, h * 512 + dc:(h + 1) * 512],
                            mybir.ActivationFunctionType.Exp, scale=SCALE)
                for h in range(HPC):
                    nc.tensor.matmul(
                        otp[h][:, dc:512],
                        vas[b][j][:, h * (HD + 1):(h + 1) * (HD + 1)],
                        pt2[:, h * 512 + dc:(h + 1) * 512],
                        start=(j == 0), stop=(j == jhi),
                        skip_group_check=True)

            pend = [(0, emit_scores(0))]
            if jhi >= 1:
                pend.append((1, emit_scores(1)))
            for j in range(2, jhi + 1):
                pend.append((j, emit_scores(j)))
                jd, scd = pend.pop(0)
                emit_exp_av(jd, scd)
            for jd, scd in pend:
                emit_exp_av(jd, scd)

            # normalize: OT2[h] = OT_aug[0:64] * bcast(1 / OT_aug[64]);
            # the partition-broadcast of the reciprocal row is a K=1 matmul
            # (ones-column outer product) — cheap on PE, keeps GPSIMD out of
            # the PSUM-slot release chain.
            for h in range(HPC):
                rec = smp.tile([1, 512], F16, tag="rec")
                nc.vector.reciprocal(rec[:], otp[h][HD:HD + 1, :])
                bcp = ps_mm.tile([HD, 512], F32, tag="mm", name="bcp")
                nc.tensor.matmul(bcp[:], ones1_t[:], rec[:],
                                 start=True, stop=True)
                bc = smp.tile([HD, 512], F16, tag="bc")
                nc.vector.tensor_copy(bc[:], bcp[:])
                nc.vector.tensor_mul(
                    ot2s[b][h * HD:(h + 1) * HD, isl], otp[h][0:HD, :], bc[:])
            emit_proj_blocks(b, range(4 * it, 4 * it + 4))

        # Emission order interleaves batch 1's QKV projection groups into
        # batch 0's (ACT-bound) attention blocks so the in-order PE queue
        # always has streaming work during exp stalls.
        emit_loads(0)
        emit_loads(1)
        for g in range(3 * NI):
            emit_qkv_group(0, g)
        emit_va(0)
        if "att" in phases:
            for it in range(NI):
                emit_att_it(0, it)
                for g in range(3 * it, 3 * (it + 1)):
                    emit_qkv_group(1, g)
            emit_va(1)
            for it in range(NI):
                emit_att_it(1, it)
        else:
            for g in range(3 * NI):
                emit_qkv_group(1, g)
            emit_va(1)

    if niter >= 1:
        for _ in range(niter):
            body()
    else:
        with tc.For_i(0, -niter, 1) as iv:
            body(iv)

    for c in reversed(ctxs):
        c.__exit__(None, None, None)


def _build(mode, niter=1, phases=("p1", "att", "proj")):
    key = (mode, niter, phases)
    if key in _BUILD_CACHE:
        return _BUILD_CACHE[key]
    nc = bacc.Bacc("TRN2", target_bir_lowering=False, debug=False,
                   num_devices=N_CORES)
    t = {}
    t["xt"] = nc.dram_tensor("xt", (B, D, S), F16, kind="ExternalInput")
    t["wqT"] = nc.dram_tensor("wqT", (D, DL), F16, kind="ExternalInput")
    t["wkT"] = nc.dram_tensor("wkT", (D, DL), F16, kind="ExternalInput")
    t["wvT"] = nc.dram_tensor("wvT", (D, DL), F16, kind="ExternalInput")
    t["pwT"] = nc.dram_tensor("pwT", (DL, D), F16, kind="ExternalInput")
    t["rpr2T"] = nc.dram_tensor("rpr2T", (128, S), F16, kind="ExternalInput")
    t["trilT"] = nc.dram_tensor("trilT", (128, 128), F32, kind="ExternalInput")
    t["ident"] = nc.dram_tensor("ident", (128, 128), F16, kind="ExternalInput")
    t["onesc"] = nc.dram_tensor("onesc", (128, 1), F16, kind="ExternalInput")
    t["ones1"] = nc.dram_tensor("ones1", (1, HD), F16, kind="ExternalInput")
    t["zeroc"] = nc.dram_tensor("zeroc", (128, 384), F16, kind="ExternalInput")
    if mode == "generic":
        t["maskT"] = nc.dram_tensor("maskT", (S, S), F32, kind="ExternalInput")
    t["y"] = nc.dram_tensor("y", (B, S, D), F16, kind="ExternalOutput")

    with tile.TileContext(nc) as tc, \
            nc.allow_low_precision(reason="float32r matmul operands (TF32-class)"):
        _emit(nc, tc, t, mode, niter, phases)
    nc.compile()
    _BUILD_CACHE[key] = (nc, t)
    return nc, t


def _prep_inputs(x, positions, causal_mask, wq, wk, wv, rpr, proj_w):
    """Host-side shard prep.  Returns (mode, per-core input maps)."""
    mask = np.asarray(causal_mask, np.float32).reshape(S, S)
    low = np.tril(np.ones((S, S), dtype=bool))
    if not mask.any():
        mode, mval = "zero", 0.0
    elif (mask[low] == 0.0).all() and np.all(mask[~low] <= -1e6) \
            and np.all(mask[~low] == mask[0, 1]):
        mode, mval = "causal", float(mask[0, 1])
    else:
        mode, mval = "generic", 0.0

    xt = np.ascontiguousarray(np.asarray(x, np.float32).transpose(0, 2, 1)).astype(np.float16)
    pos = np.asarray(positions).astype(np.int64)
    rpr_g = np.asarray(rpr, np.float32)[pos]  # (B, S, HD)
    rpr2 = np.ascontiguousarray(
        rpr_g.transpose(0, 2, 1)).reshape(B * HD, S).astype(np.float16)  # (128, S)
    jj = np.arange(128)[:, None]
    ii = np.arange(128)[None, :]
    trilT = np.where(jj <= ii, 0.0, mval).astype(np.float32)
    ident = np.eye(128, dtype=np.float32)
    maskT = np.ascontiguousarray(mask.T) if mode == "generic" else None

    wq = np.asarray(wq, np.float32)
    wk = np.asarray(wk, np.float32)
    wv = np.asarray(wv, np.float32)
    pw = np.asarray(proj_w, np.float32)

    in_maps = []
    for c in range(N_CORES):
        rs = slice(c * DL, (c + 1) * DL)
        m = {
            "xt": xt,
            "wqT": np.ascontiguousarray(wq[rs, :].T).astype(np.float16),
            "wkT": np.ascontiguousarray(wk[rs, :].T).astype(np.float16),
            "wvT": np.ascontiguousarray(wv[rs, :].T).astype(np.float16),
            "pwT": np.ascontiguousarray(pw[:, rs].T).astype(np.float16),
            "rpr2T": rpr2,
            "trilT": trilT,
            "ident": ident.astype(np.float16),
            "onesc": np.ones((128, 1), np.float16),
            "ones1": np.ones((1, HD), np.float16),
            "zeroc": np.zeros((128, 384), np.float16),
        }
        if maskT is not None:
            m["maskT"] = maskT
        in_maps.append(m)
    return mode, in_maps


def kernel(x, positions, causal_mask, wq, wk, wv, rpr, proj_w, proj_b,
           _niter=1, _phases=("p1", "att", "proj"), **_ignored):
    mode, in_maps = _prep_inputs(x, positions, causal_mask, wq, wk, wv, rpr,
                                 proj_w)
    nc, _ = _build(mode, _niter, _phases)
    res = run_bass_kernel_spmd(nc, in_maps, core_ids=list(range(N_CORES)))
    out = np.zeros((B, S, D), dtype=np.float32)
    for r in res.results:
        out += r["y"].astype(np.float32)
    out += np.asarray(proj_b, np.float32)[None, None, :]
    return out
